# revision 5
# baseline (speedup 1.0000x reference)
"""Trainium2 Bass kernel for nn_DWTModelSimple.

The reference computes a 2-level orthonormal Haar DWT and immediately
inverts it with the exact same cached high-frequency subbands.  Per 2x2
block the inverse butterfly reconstructs a,b,c,d exactly, so
idwt(idwt(dwt(dwt(x)))) == x: the whole module is the identity map.
The float32 reference deviates from x only by its own rounding noise
(~6e-8 norm-relative), so the memory-roofline implementation is a
straight HBM->HBM copy, data-parallel over the batch dimension
(batch 32 -> 4 per core across 8 NeuronCores; 12.58 MB per core).

The copy streams DRAM->DRAM through one HWDGE ring (SP) feeding all 16
SDMA engines, one queue per engine.  Profiling findings this build
encodes (all verified from NTFF profiles on the axon trn2 cores):

* Descriptor i of a PDMA2D trigger is serviced by SDMA engine i%16,
  restarting at engine 0 for every trigger (verified from per-queue
  byte counts).
* One queue per engine is slightly faster than two (single-ring
  ~20.8-21.1 GB/s per engine vs ~20.5 effective with SP+ACT rings,
  which round-robin packets per engine), needs only one completion
  wait, and makes SP - whose slot in the NRT exit-barrier chain is
  late - the last finisher, which shaves a serialization hop off the
  exit ABI.
* SDMA engine 15 runs ~0.85x the rate of engines 0-14 in some runs
  (variable run to run) and starts ~1-2 us late; at equal load it
  finished ~7 us after the pack, directly delaying the completion
  semaphore.  Engine 14 starts late too (last in round-robin order).
  The layout gives engine 15 ~0.81x and engine 14 ~0.94x of a regular
  engine's bytes: under-loading is cheap insurance (the pack absorbs
  <1% extra) while an engine-15 tail costs multiple us.
* The AP optimizer merges contiguous rows and re-splits 16-wide, so
  sub-16-descriptor triggers must use stride-2 interleaved row pairs
  (triggers of [r1 rows, step 2] + [r2 rows, step 2], r2 in
  {r1, r1-1}, tiling a (r1+r2)-row region) whose APs cannot be merged.
  Contiguous base regions are left mergeable on purpose: they lower to
  48 KB descriptors spread evenly 16-wide.
* HWDGE posts exactly 16 completion increments per trigger (one per
  SDMA engine, regardless of the trigger's descriptor count), so the
  ring waits for 16 x n_triggers on its semaphore.
* The measured exec window starts at the first DMA trigger (the NEFF
  entry ABI before it is excluded by the profiler's first-useful-time)
  and ends at the last engine halt; the NRT-injected exit ABI (two
  $S[2] barriers + a 5-way-split clear sweep of semaphores 3..255 +
  NOTIFY) is a fixed ~7 us tail patched into the instruction streams
  at load time - it is not in the walrus-emitted engine binaries and
  cannot be trimmed from the kernel side.

The module is built straight-line and then IR-spliced so the DMA
trigger instructions execute ahead of bass's init-barrier run (the
stream launches the moment the NEFF entry sequence ends).  A guarded
fallback rebuilds the plain Block form if the preamble structure ever
changes.

Measured on 8 axon trn2 cores (best of 3, NTFF profile of core 0):
even-split dual-ring baseline 53.3 us; weighted dual-ring 48.2 us;
this single-ring weighted layout 47.9 us (reps 47.9/48.0/48.2), i.e.
12.58 MB payload streaming at ~650 GB/s HBM read+write plus the fixed
~7 us exit ABI.
"""

import numpy as np

import concourse.bass as bass
import concourse.mybir as mybir
from concourse.bass_utils import run_bass_kernel_spmd

N_CORES = 8
B, C, H, W = 32, 3, 512, 512
B_PER_CORE = B // N_CORES
ELEMS_PER_CORE = B_PER_CORE * C * H * W  # 3,145,728

QUANT = 6144                      # elems per row (24,576 B descriptors)
N_ROWS = ELEMS_PER_CORE // QUANT  # 512
P = N_ROWS
FREE = QUANT

# Ring layout: contiguous base regions (rows; multiples of 32 so they
# lower to evenly-spread 48 KB descriptors), stride-2 trim pairs (r1, r2)
# loading only engines 0..r1-1 / 0..r2-1, and a remainder region (spread
# 16-wide as small descriptors).  Engine quanta: e0-13: 32, e14: 30,
# e15: 26 (+ rem), i.e. 780 / 732 / 636 KB.
RING_CFG = dict(
    base=(64, 64, 64, 64, 64, 64),
    pairs=((16, 15), (15, 14), (16, 15), (15, 14)),
    rem=8,
)
assert (
    sum(RING_CFG["base"])
    + sum(r1 + r2 for r1, r2 in RING_CFG["pairs"])
    + RING_CFG["rem"]
    == N_ROWS
)

_cached_nc = None


def _emit_ring(eng, sem, x, y, r0: int, cfg) -> tuple[int, int]:
    """Emit the ring's triggers starting at row r0. Returns (next_row,
    n_triggers)."""
    n = 0
    for rows in cfg["base"]:
        eng.dma_start(y[r0 : r0 + rows, :], x[r0 : r0 + rows, :]).then_inc(sem, 16)
        r0 += rows
        n += 1
    for r1, r2 in cfg["pairs"]:
        assert r1 - 1 <= r2 <= r1
        eng.dma_start(
            y[r0 : r0 + 2 * r1 - 1 : 2, :], x[r0 : r0 + 2 * r1 - 1 : 2, :]
        ).then_inc(sem, 16)
        eng.dma_start(
            y[r0 + 1 : r0 + 2 * r2 : 2, :], x[r0 + 1 : r0 + 2 * r2 : 2, :]
        ).then_inc(sem, 16)
        r0 += r1 + r2
        n += 2
    rem = cfg["rem"]
    if rem:
        eng.dma_start(y[r0 : r0 + rem, :], x[r0 : r0 + rem, :]).then_inc(sem, 16)
        r0 += rem
        n += 1
    return r0, n


def _build_nc_spliced() -> bass.Bass:
    """Straight-line build + IR splice: hoist the DMA trigger instructions
    ahead of bass's init-barrier run so the stream launches as soon as the
    NEFF entry sequence finishes.  The completion wait stays at the end of
    the engine's stream."""
    SP = mybir.EngineType.SP

    nc = bass.Bass()
    main = nc.m.functions[0].blocks[0]
    assert main.name == "main", main.name
    pre_n = len(main.instructions)

    x = nc.dram_tensor("x", [P, FREE], mybir.dt.float32, kind="ExternalInput")
    y = nc.dram_tensor("y", [P, FREE], mybir.dt.float32, kind="ExternalOutput")

    with nc.semaphore("sem_sp") as sem_sp:
        r0, n_sp = _emit_ring(nc.sync, sem_sp, x, y, 0, RING_CFG)
        assert r0 == N_ROWS, r0
        # wait emitted last so the splice below can separate it
        nc.sync.wait_ge(sem_sp, 16 * n_sp)

    insts = main.instructions
    pre, user = list(insts[:pre_n]), list(insts[pre_n:])
    assert all(i.engine == SP for i in user)

    waits = [i for i in user if isinstance(i, mybir.InstEventSemaphore)]
    assert len(waits) == 1, [type(i).__name__ for i in user]
    sp_wait = waits[0]
    sp_trig = [i for i in user if i is not sp_wait]

    def splice_point(eng):
        # index of the first instruction of the engine's trailing
        # Drain/EventSemaphore run (the init barrier) in the preamble
        idxs = [k for k, i in enumerate(pre) if i.engine == eng]
        assert idxs
        j = len(idxs)
        while j > 0 and isinstance(
            pre[idxs[j - 1]], (mybir.InstDrain, mybir.InstEventSemaphore)
        ):
            j -= 1
        assert j < len(idxs), "no barrier run found"
        return idxs[j]

    p_sp = splice_point(SP)
    new = []
    for k, inst in enumerate(pre):
        if k == p_sp:
            new.extend(sp_trig)
        new.append(inst)
    new.append(sp_wait)
    assert len(new) == len(insts), (len(new), len(insts))
    insts[:] = new
    return nc


def _build_nc_plain() -> bass.Bass:
    nc = bass.Bass()
    x = nc.dram_tensor("x", [P, FREE], mybir.dt.float32, kind="ExternalInput")
    y = nc.dram_tensor("y", [P, FREE], mybir.dt.float32, kind="ExternalOutput")

    with nc.semaphore("sem_sp") as sem_sp, nc.Block() as block:

        @block.sync
        def _(sync):
            _, n = _emit_ring(sync, sem_sp, x, y, 0, RING_CFG)
            sync.wait_ge(sem_sp, 16 * n)

    return nc


def _build_nc() -> bass.Bass:
    try:
        return _build_nc_spliced()
    except Exception:
        # Fall back to the long-validated Block form if the preamble
        # structure ever changes under the splice's assertions.
        return _build_nc_plain()


def get_nc() -> bass.Bass:
    global _cached_nc
    if _cached_nc is None:
        _cached_nc = _build_nc()
    return _cached_nc


def kernel(x: np.ndarray) -> np.ndarray:
    x = np.ascontiguousarray(x, dtype=np.float32)
    assert x.shape == (B, C, H, W), x.shape

    in_maps = [
        {"x": x[i * B_PER_CORE : (i + 1) * B_PER_CORE].reshape(P, FREE)}
        for i in range(N_CORES)
    ]
    try:
        res = run_bass_kernel_spmd(get_nc(), in_maps, core_ids=list(range(N_CORES)))
    except Exception:
        # One retry for transient runtime hiccups (e.g. a core recovering
        # from a previous process's interrupted run).
        res = run_bass_kernel_spmd(get_nc(), in_maps, core_ids=list(range(N_CORES)))
    return np.concatenate(
        [res.results[i]["y"].reshape(B_PER_CORE, C, H, W) for i in range(N_CORES)],
        axis=0,
    )


# revision 6
# speedup vs baseline: 1.0247x; 1.0247x over previous
"""Trainium2 Bass kernel for nn_DWTModelSimple.

The reference computes a 2-level orthonormal Haar DWT and immediately
inverts it with the exact same cached high-frequency subbands.  Per 2x2
block the inverse butterfly reconstructs a,b,c,d exactly, so
idwt(idwt(dwt(dwt(x)))) == x: the whole module is the identity map.
The float32 reference deviates from x only by its own rounding noise
(~6e-8 norm-relative), so the memory-roofline implementation is a
straight HBM->HBM copy, data-parallel over the batch dimension
(batch 32 -> 4 per core across 8 NeuronCores; 12.58 MB per core).

The copy streams DRAM->DRAM through one HWDGE ring (SP) feeding all 16
SDMA engines, one queue per engine.  Profiling findings this build
encodes (all verified from NTFF profiles on the axon trn2 cores):

* Descriptor i of a PDMA2D trigger is serviced by SDMA engine i%16,
  restarting at engine 0 for every trigger (verified from per-queue
  byte counts).
* One queue per engine is slightly faster than two (single-ring
  ~20.8-21.1 GB/s per engine vs ~20.5 effective with SP+ACT rings,
  which round-robin packets per engine), needs only one completion
  wait, and makes SP - whose slot in the NRT exit-barrier chain is
  late - the last finisher, which shaves a serialization hop off the
  exit ABI.
* SDMA engine 15 runs ~0.85x the rate of engines 0-14 in some runs
  (variable run to run) and starts ~1-2 us late; at equal load it
  finished ~7 us after the pack, directly delaying the completion
  semaphore.  Engine 14 starts late too (last in round-robin order).
  The layout gives engine 15 ~0.81x and engine 14 ~0.94x of a regular
  engine's bytes: under-loading is cheap insurance (the pack absorbs
  <1% extra) while an engine-15 tail costs multiple us.
* The AP optimizer merges contiguous rows and re-splits 16-wide, so
  sub-16-descriptor triggers must use stride-2 interleaved row pairs
  (triggers of [r1 rows, step 2] + [r2 rows, step 2], r2 in
  {r1, r1-1}, tiling a (r1+r2)-row region) whose APs cannot be merged.
  Contiguous base regions are left mergeable on purpose: they lower to
  48 KB descriptors spread evenly 16-wide.
* HWDGE posts exactly 16 completion increments per trigger (one per
  SDMA engine, regardless of the trigger's descriptor count), so the
  ring waits for 16 x n_triggers on its semaphore.
* The measured exec window starts at the first DMA trigger (the NEFF
  entry ABI before it is excluded by the profiler's first-useful-time)
  and ends at the last engine halt; the NRT-injected exit ABI (two
  $S[2] barriers + a 5-way-split clear sweep of semaphores 3..255 +
  NOTIFY) is a fixed ~7 us tail patched into the instruction streams
  at load time - it is not in the walrus-emitted engine binaries and
  cannot be trimmed from the kernel side.

The module is built straight-line and then IR-spliced so the DMA
trigger instructions execute ahead of bass's init-barrier run (the
stream launches the moment the NEFF entry sequence ends).  A guarded
fallback rebuilds the plain Block form if the preamble structure ever
changes.

Measured on 8 axon trn2 cores (best of 3, NTFF profile of core 0):
even-split dual-ring baseline 53.3 us; weighted dual-ring 48.2 us;
this single-ring weighted layout 47.9 us (reps 47.9/48.0/48.2), i.e.
12.58 MB payload streaming at ~650 GB/s HBM read+write plus the fixed
~7 us exit ABI.
"""

import numpy as np

import concourse.bass as bass
import concourse.mybir as mybir
from concourse.bass_utils import run_bass_kernel_spmd

N_CORES = 8
B, C, H, W = 32, 3, 512, 512
B_PER_CORE = B // N_CORES
ELEMS_PER_CORE = B_PER_CORE * C * H * W  # 3,145,728

QUANT = 6144                      # elems per row (24,576 B descriptors)
N_ROWS = ELEMS_PER_CORE // QUANT  # 512
P = N_ROWS
FREE = QUANT

# Ring layout: contiguous base regions (rows; multiples of 32 so they
# lower to evenly-spread 48 KB descriptors), stride-2 trim pairs (r1, r2)
# loading only engines 0..r1-1 / 0..r2-1, and a remainder region (spread
# 16-wide as small descriptors).  Engine quanta: e0-13: 32, e14: 30,
# e15: 26 (+ rem), i.e. 780 / 732 / 636 KB.
# The leading 32-row chunk (16 descriptors) rings the first doorbell
# ~0.3 us sooner than a 64-row chunk would, starting all engines earlier.
RING_CFG = dict(
    base=(32, 64, 64, 64, 64, 64, 32),
    pairs=((16, 15), (15, 14), (16, 15), (15, 14)),
    rem=8,
)
assert (
    sum(RING_CFG["base"])
    + sum(r1 + r2 for r1, r2 in RING_CFG["pairs"])
    + RING_CFG["rem"]
    == N_ROWS
)

_cached_nc = None


def _emit_ring(eng, sem, x, y, r0: int, cfg) -> tuple[int, int]:
    """Emit the ring's triggers starting at row r0. Returns (next_row,
    n_triggers)."""
    n = 0
    for rows in cfg["base"]:
        eng.dma_start(y[r0 : r0 + rows, :], x[r0 : r0 + rows, :]).then_inc(sem, 16)
        r0 += rows
        n += 1
    for r1, r2 in cfg["pairs"]:
        assert r1 - 1 <= r2 <= r1
        eng.dma_start(
            y[r0 : r0 + 2 * r1 - 1 : 2, :], x[r0 : r0 + 2 * r1 - 1 : 2, :]
        ).then_inc(sem, 16)
        eng.dma_start(
            y[r0 + 1 : r0 + 2 * r2 : 2, :], x[r0 + 1 : r0 + 2 * r2 : 2, :]
        ).then_inc(sem, 16)
        r0 += r1 + r2
        n += 2
    rem = cfg["rem"]
    if rem:
        eng.dma_start(y[r0 : r0 + rem, :], x[r0 : r0 + rem, :]).then_inc(sem, 16)
        r0 += rem
        n += 1
    return r0, n


def _build_nc_spliced() -> bass.Bass:
    """Straight-line build + IR splice: hoist the DMA trigger instructions
    ahead of bass's init-barrier run so the stream launches as soon as the
    NEFF entry sequence finishes.  The completion wait stays at the end of
    the engine's stream."""
    SP = mybir.EngineType.SP

    nc = bass.Bass()
    main = nc.m.functions[0].blocks[0]
    assert main.name == "main", main.name
    pre_n = len(main.instructions)

    x = nc.dram_tensor("x", [P, FREE], mybir.dt.float32, kind="ExternalInput")
    y = nc.dram_tensor("y", [P, FREE], mybir.dt.float32, kind="ExternalOutput")

    with nc.semaphore("sem_sp") as sem_sp:
        r0, n_sp = _emit_ring(nc.sync, sem_sp, x, y, 0, RING_CFG)
        assert r0 == N_ROWS, r0
        # wait emitted last so the splice below can separate it
        nc.sync.wait_ge(sem_sp, 16 * n_sp)

    insts = main.instructions
    pre, user = list(insts[:pre_n]), list(insts[pre_n:])
    assert all(i.engine == SP for i in user)

    waits = [i for i in user if isinstance(i, mybir.InstEventSemaphore)]
    assert len(waits) == 1, [type(i).__name__ for i in user]
    sp_wait = waits[0]
    sp_trig = [i for i in user if i is not sp_wait]

    def splice_point(eng):
        # index of the first instruction of the engine's trailing
        # Drain/EventSemaphore run (the init barrier) in the preamble
        idxs = [k for k, i in enumerate(pre) if i.engine == eng]
        assert idxs
        j = len(idxs)
        while j > 0 and isinstance(
            pre[idxs[j - 1]], (mybir.InstDrain, mybir.InstEventSemaphore)
        ):
            j -= 1
        assert j < len(idxs), "no barrier run found"
        return idxs[j]

    p_sp = splice_point(SP)
    new = []
    for k, inst in enumerate(pre):
        if k == p_sp:
            new.extend(sp_trig)
        new.append(inst)
    new.append(sp_wait)
    assert len(new) == len(insts), (len(new), len(insts))
    insts[:] = new
    return nc


def _build_nc_plain() -> bass.Bass:
    nc = bass.Bass()
    x = nc.dram_tensor("x", [P, FREE], mybir.dt.float32, kind="ExternalInput")
    y = nc.dram_tensor("y", [P, FREE], mybir.dt.float32, kind="ExternalOutput")

    with nc.semaphore("sem_sp") as sem_sp, nc.Block() as block:

        @block.sync
        def _(sync):
            _, n = _emit_ring(sync, sem_sp, x, y, 0, RING_CFG)
            sync.wait_ge(sem_sp, 16 * n)

    return nc


def _build_nc() -> bass.Bass:
    try:
        return _build_nc_spliced()
    except Exception:
        # Fall back to the long-validated Block form if the preamble
        # structure ever changes under the splice's assertions.
        return _build_nc_plain()


def get_nc() -> bass.Bass:
    global _cached_nc
    if _cached_nc is None:
        _cached_nc = _build_nc()
    return _cached_nc


def kernel(x: np.ndarray) -> np.ndarray:
    x = np.ascontiguousarray(x, dtype=np.float32)
    assert x.shape == (B, C, H, W), x.shape

    in_maps = [
        {"x": x[i * B_PER_CORE : (i + 1) * B_PER_CORE].reshape(P, FREE)}
        for i in range(N_CORES)
    ]
    try:
        res = run_bass_kernel_spmd(get_nc(), in_maps, core_ids=list(range(N_CORES)))
    except Exception:
        # One retry for transient runtime hiccups (e.g. a core recovering
        # from a previous process's interrupted run).
        res = run_bass_kernel_spmd(get_nc(), in_maps, core_ids=list(range(N_CORES)))
    return np.concatenate(
        [res.results[i]["y"].reshape(B_PER_CORE, C, H, W) for i in range(N_CORES)],
        axis=0,
    )
